# revision 3
# baseline (speedup 1.0000x reference)
"""LIF (leaky integrate-and-fire) forward kernel for Trainium2, 8 NeuronCores.

Reference semantics (per element, scan over T):
    u = LAM * u + x_t
    o_t = (u - THRESHOLD > 0) ? 1.0 : 0.0
    u = u - o_t

Sharding: pure data parallel over batch B=16 -> 2 samples per core.
Per-core layout: C=128 channels on SBUF partitions, free dim = (b_loc, H*W).
All arithmetic is IEEE fp32 on the DVE -> bit-identical to the jax CPU
reference (mul-by-0.5 is exact; add/sub correctly rounded; compare exact).
"""

import numpy as np

B, T, C, HW = 16, 16, 128, 1024  # HW = 32*32
N_CORES = 8
B_LOC = B // N_CORES  # 2
CHUNK = 4  # timesteps per DMA chunk (4 MiB per transfer)
LAM = 0.5
THRESHOLD = 1.0

_CACHE = {}


def _build():
    import concourse.tile as tile
    import concourse.mybir as mybir
    from concourse import bacc

    nc = bacc.Bacc(
        "TRN2",
        target_bir_lowering=False,
        debug=False,
        enable_asserts=False,
        num_devices=N_CORES,
    )
    f32 = mybir.dt.float32
    Alu = mybir.AluOpType

    x_dram = nc.dram_tensor("x", [B_LOC, T, C, HW], f32, kind="ExternalInput")
    o_dram = nc.dram_tensor("o", [B_LOC, T, C, HW], f32, kind="ExternalOutput")
    # partition dim = C; free iteration (t, b, f) with f contiguous in DRAM
    x = x_dram.ap().rearrange("b t c f -> c t b f")
    o = o_dram.ap().rearrange("b t c f -> c t b f")

    with tile.TileContext(nc) as tc:
        with (
            tc.tile_pool(name="xp", bufs=2) as xp,
            tc.tile_pool(name="op", bufs=2) as op_,
            tc.tile_pool(name="up", bufs=1) as up,
        ):
            u = up.tile([C, B_LOC, HW], f32)
            nc.vector.memset(u[:], 0.0)
            for t0 in range(0, T, CHUNK):
                xt = xp.tile([C, CHUNK, B_LOC, HW], f32)
                for b in range(B_LOC):
                    # 3-dim AP limit per DMA: split per sample (2 MiB each)
                    nc.sync.dma_start(
                        xt[:, :, b], x[:, t0 : t0 + CHUNK, b]
                    )
                ot = op_.tile([C, CHUNK, B_LOC, HW], f32)
                for ti in range(CHUNK):
                    xs = xt[:, ti]
                    os_ = ot[:, ti]
                    # u = (u * LAM) + x_t
                    nc.vector.scalar_tensor_tensor(
                        u[:], u[:], LAM, xs, op0=Alu.mult, op1=Alu.add
                    )
                    # o_t = (u > THRESHOLD) as 1.0/0.0
                    nc.vector.tensor_scalar(
                        os_, u[:], THRESHOLD, None, op0=Alu.is_gt
                    )
                    # u = u - o_t
                    nc.vector.tensor_tensor(u[:], u[:], os_, op=Alu.subtract)
                for b in range(B_LOC):
                    nc.sync.dma_start(
                        o[:, t0 : t0 + CHUNK, b], ot[:, :, b]
                    )
    nc.compile()
    return nc


def _get_nc():
    if "nc" not in _CACHE:
        _CACHE["nc"] = _build()
    return _CACHE["nc"]


def kernel(x_seq, noise=None, **_ignored):
    from concourse import bass_utils

    nc = _get_nc()
    x = np.ascontiguousarray(np.asarray(x_seq), dtype=np.float32).reshape(B, T, C, HW)
    in_maps = [
        {"x": x[i * B_LOC : (i + 1) * B_LOC]} for i in range(N_CORES)
    ]
    res = bass_utils.run_bass_kernel_spmd(
        nc, in_maps, core_ids=list(range(N_CORES))
    )
    out = np.concatenate([r["o"] for r in res.results], axis=0)
    return out.reshape(B, T, C, 32, 32)
